# revision 14
# baseline (speedup 1.0000x reference)
"""DeepSeekMOE grouped masked GEMM kernel for 8 Trainium2 NeuronCores.

Per group g: out_ug = x_ug[g] @ w_ug[g].T [32, 2816], out_dn = x_dn[g] @
w_dn[g].T [32, 4096]; rows >= masked_m[g] are zero (x rows zeroed on host);
output [8, 32, 6912] = concat(out_ug, out_dn).

Memory-bound: the weight stream dominates. Two levers over the f32
expert-parallel baseline (193 us, exactly the 358 GB/s/core DMA roofline):

1. bf16 weights + activations (host-cast): halves DMA bytes. Measured
   l2_rel 2.3e-3 vs the f32 reference, far under the 2e-2 gate.
2. Masked-group skip + rebalance ("bal" mode): groups with masked_m[g] == 0
   contribute nothing, so their weights are never read. The remaining nA
   groups' weight columns are split into 8 n-slices each (ug 352, dn 512
   wide); the 8*nA slices are dealt nA-per-core so every core streams
   nA/8 of the active bytes. Per-slice x is replicated into an SBUF-layout
   image on host, keeping the device program uniform across cores (SPMD:
   one program, per-core data).

Matmuls are bf16 x bf16 -> f32 PSUM, accumulated over k; n-slices sit in
512-col (2 KB) PSUM bank slots so partial-width ug slices stay bank-aligned.
"""
import numpy as np
import ml_dtypes

import concourse.bass as bass
import concourse.bacc as bacc
import concourse.mybir as mybir
import concourse.tile as tile
from concourse.bass_utils import run_bass_kernel_spmd

G, M = 8, 32
K_UG, N_UG = 4096, 2816
K_DN, N_DN = 1408, 4096
N_OUT = N_UG + N_DN
P = 128
KC_UG = K_UG // P  # 32 k-chunks
KC_DN = K_DN // P  # 11 k-chunks
SU = N_UG // 8  # 352: ug n-slice width
SD = N_DN // 8  # 512: dn n-slice width
SLAB = 2  # k-chunks per weight DMA

f32 = mybir.dt.float32
bf16 = mybir.dt.bfloat16
nbf16 = ml_dtypes.bfloat16

TRACE = False  # NTFF tracing unavailable over axon; timing lives in bench.py
_cache = {}


def _slabs_of(kc_total, slab):
    slabs = []
    c0 = 0
    while c0 < kc_total:
        slen = min(slab, kc_total - c0)
        slabs.append((c0, slen))
        c0 += slen
    return slabs


def _build_program(reps=1, nA=7, xrep=None, slab=SLAB, wbufs=4, no_dma=False,
                   no_pe=False):
    """nA active groups' n-slices dealt nA-per-core.

    xrep = distinct x chunks in the per-core x image: nA when cores mix
    groups (balanced mode), 1 when every slice is the core's own group
    (expert-parallel, nA=8) so x needn't be replicated per slice.
    reps>1 wraps the body in a HW loop (bench-only, for slope timing).
    no_dma / no_pe are bench-only probes isolating PE / DMA time.
    """
    if xrep is None:
        xrep = 1 if nA == 8 else nA
    nc = bacc.Bacc("TRN2", target_bir_lowering=False, debug=False)

    NTU = nA * SU  # ug cols per core
    NTD = nA * SD  # dn cols per core
    xu = nc.dram_tensor("xu", [P, xrep * KC_UG * M], bf16, kind="ExternalInput")
    xd = nc.dram_tensor("xd", [P, xrep * KC_DN * M], bf16, kind="ExternalInput")
    wu = nc.dram_tensor("wu", [K_UG, NTU], bf16, kind="ExternalInput")
    wd = nc.dram_tensor("wd", [K_DN, NTD], bf16, kind="ExternalInput")
    out = nc.dram_tensor("out", [M, NTU + NTD], f32, kind="ExternalOutput")

    import contextlib

    with contextlib.ExitStack() as stack:
        tc = stack.enter_context(tile.TileContext(nc))
        wpool = stack.enter_context(tc.tile_pool(name="w", bufs=wbufs))
        misc = stack.enter_context(tc.tile_pool(name="misc", bufs=1))
        psum = stack.enter_context(tc.tile_pool(name="psum", bufs=1, space="PSUM"))
        if reps > 1:
            stack.enter_context(tc.For_i(0, reps, 1))
        if True:
            # Stationary x images, already host-packed in SBUF layout:
            # chunk j, k-chunk c at columns [((j % xrep)*KC + c)*M, ...+M).
            xu_t = misc.tile([P, xrep * KC_UG * M], bf16, tag="xu")
            nc.sync.dma_start(xu_t[:], xu[:])
            xd_t = misc.tile([P, xrep * KC_DN * M], bf16, tag="xd")
            nc.sync.dma_start(xd_t[:], xd[:])

            o_t = misc.tile([M, NTU + NTD], f32, tag="o")
            for wd_d, x_t, kc_tot, S, ntot, o_off, wtag in (
                (wu, xu_t, KC_UG, SU, NTU, 0, "wu"),
                (wd, xd_t, KC_DN, SD, NTD, NTU, "wd"),
            ):
                acc = None if no_pe else psum.tile([M, nA * 512], f32, tag="acc")
                w_src = wd_d[:].rearrange("(c k) n -> k c n", k=P)
                for c0, slen in _slabs_of(kc_tot, slab):
                    w_t = wpool.tile([P, slen * ntot], bf16, tag="w")
                    if not no_dma:
                        nc.sync.dma_start(
                            w_t[:, : slen * ntot].rearrange(
                                "k (c n) -> k c n", c=slen
                            ),
                            w_src[:, c0 : c0 + slen, :],
                        )
                    if no_pe:
                        continue
                    for c in range(slen):
                        kc = c0 + c
                        for j in range(nA):
                            jx = j % xrep
                            nc.tensor.matmul(
                                acc[:, j * 512 : j * 512 + S],
                                x_t[:, (jx * kc_tot + kc) * M : (jx * kc_tot + kc + 1) * M],
                                w_t[:, c * ntot + j * S : c * ntot + (j + 1) * S],
                                start=(kc == 0),
                                stop=(kc == kc_tot - 1),
                            )
                if no_pe:
                    continue
                for j in range(nA):
                    nc.vector.tensor_copy(
                        o_t[:, o_off + j * S : o_off + (j + 1) * S],
                        acc[:, j * 512 : j * 512 + S],
                    )
            if no_pe:
                # keep `out` written so the verifier sees a writer
                # (bench-only probe; values are garbage)
                wu_ = min(NTU, xrep * KC_UG * M)
                wd_ = min(NTD, xrep * KC_DN * M)
                nc.vector.tensor_copy(o_t[:, :wu_], xu_t[:M, :wu_])
                nc.vector.tensor_copy(o_t[:, NTU : NTU + wd_], xd_t[:M, :wd_])
            nc.sync.dma_start(out[:], o_t[:])

    nc.compile()
    return nc


def _plan(masked_m):
    """Slice assignment: plan[c][j] = (group, slice-in-group) for core c's
    j-th n-slice. nA=8: expert-parallel (core c owns group c, xrep=1).
    nA<8: slices dealt round-robin, global slice s = nA*c + j ->
    (active[s // 8], s % 8), xrep=nA."""
    active = [g for g in range(G) if int(masked_m[g]) > 0]
    nA = len(active)
    if nA == 8:
        return active, 8, 1, [[(c, j) for j in range(8)] for c in range(G)]
    plan = [
        [(active[(nA * c + j) // 8], (nA * c + j) % 8) for j in range(nA)]
        for c in range(G)
    ]
    return active, nA, nA, plan


def _prepare_in_maps(inputs):
    x_ug = np.asarray(inputs["x_ug"], dtype=np.float32)
    w_ug = np.asarray(inputs["w_ug"], dtype=np.float32)
    x_dn = np.asarray(inputs["x_dn"], dtype=np.float32)
    w_dn = np.asarray(inputs["w_dn"], dtype=np.float32)
    masked_m = np.asarray(inputs["masked_m"])
    active, nA, xrep, plan = _plan(masked_m)
    if nA == 0:
        return []

    row = np.arange(M)
    valid = (row[None, :] < masked_m[:, None]).astype(np.float32)[:, :, None]
    # x SBUF images per group: [M, K] -> [P, KC, M] (partition, k-chunk, m)
    def ximg(x, KC):
        xm = (x * valid).astype(nbf16)  # [G, M, K]
        return np.ascontiguousarray(
            xm.transpose(0, 2, 1).reshape(G, KC, P, M).transpose(0, 2, 1, 3)
        )  # [G, P, KC, M]

    xiu = ximg(x_ug, KC_UG)
    xid = ximg(x_dn, KC_DN)
    # weight [K, N] images per group (bf16)
    wtu = np.ascontiguousarray(w_ug.transpose(0, 2, 1).astype(nbf16))
    wtd = np.ascontiguousarray(w_dn.transpose(0, 2, 1).astype(nbf16))

    in_maps = []
    for c in range(G):
        gs = [plan[c][j][0] for j in range(xrep)]
        in_maps.append(
            {
                "xu": xiu[gs].transpose(1, 0, 2, 3).reshape(P, xrep * KC_UG * M),
                "xd": xid[gs].transpose(1, 0, 2, 3).reshape(P, xrep * KC_DN * M),
                "wu": np.concatenate(
                    [wtu[g][:, r * SU : (r + 1) * SU] for g, r in plan[c]], axis=1
                ),
                "wd": np.concatenate(
                    [wtd[g][:, r * SD : (r + 1) * SD] for g, r in plan[c]], axis=1
                ),
            }
        )
    return in_maps


def _assemble(res, inputs):
    masked_m = np.asarray(inputs["masked_m"])
    active, nA, xrep, plan = _plan(masked_m)
    full = np.zeros((G, M, N_OUT), dtype=np.float32)
    for c in range(G):
        o = res.results[c]["out"]
        for j, (g, r) in enumerate(plan[c]):
            full[g, :, r * SU : (r + 1) * SU] = o[:, j * SU : (j + 1) * SU]
            full[g, :, N_UG + r * SD : N_UG + (r + 1) * SD] = o[
                :, nA * SU + j * SD : nA * SU + (j + 1) * SD
            ]
    return full


def kernel(x_ug, w_ug, x_dn, w_dn, masked_m):
    inputs = dict(x_ug=x_ug, w_ug=w_ug, x_dn=x_dn, w_dn=w_dn, masked_m=masked_m)
    masked_m = np.asarray(masked_m)
    active, nA, xrep, plan = _plan(masked_m)
    if nA == 0:
        return np.zeros((G, M, N_OUT), dtype=np.float32)
    key = ("nc", nA, xrep)
    if key not in _cache:
        _cache[key] = _build_program(nA=nA, xrep=xrep)
    nc = _cache[key]

    in_maps = _prepare_in_maps(inputs)

    res = None
    for attempt in range(3):
        try:
            res = run_bass_kernel_spmd(
                nc, in_maps, core_ids=list(range(G)), trace=TRACE
            )
            break
        except Exception:
            if attempt == 2:
                raise
            # Transient NRT/device failures: reset jax backends and retry.
            import time

            try:
                import jax

                jax.clear_caches()
                import jax.extend.backend as _jb

                _jb.clear_backends()
            except Exception:
                pass
            time.sleep(20.0 * (attempt + 1))
    if TRACE:
        _cache["last_result"] = res
    return _assemble(res, inputs)
